# revision 2
# baseline (speedup 1.0000x reference)
"""Trainium2 Bass kernel for nn_BasalGangliaActor_48052094108013.

The reference module is a chain of spikingjelly LIF neurons
(PFC -> {D1,D2} -> GPe -> STN -> GPi -> Thal -> PMC) scanned over T
timesteps, returning (argmax(pmc_spike_totals), pmc_spike_totals).

Exact fp32 analysis of the reference (verified bit-exactly against it):

1. ``TAU = 1.00000000001`` is a python float that JAX weakly types to
   float32, where it rounds to exactly 1.0.  The LIF membrane update
   ``v = v + (x - v)/TAU`` therefore becomes ``v = v + (x - v)``: every
   membrane tracks its input current to within ~2.4e-7 (two fp32
   roundings), and a hard reset (V_RESET=0) clears it exactly.
2. The surrogate-gradient spike ``sg + stop_grad(hard - sg)`` is
   *bit-exactly* ``hard`` in the forward pass: for v < v_th it is
   ``sg - sg = 0``; for v >= v_th, ``sg = sigmoid(4(v-1)) in [0.5, 1]``
   so Sterbenz's lemma makes ``1 - sg`` exact and ``sg + (1 - sg) = 1``.
   So spikes are exactly {0.0, 1.0} and a neuron spikes iff its
   (current-tracking) membrane is >= 1.0.
3. The output path is decided by hardcoded constants with margins of
   >= 0.05 from the 1.0 threshold — 70x larger than the worst-case
   fp32 tracking error — so it is input-independent:
     - STN channels 1-3: currents {1.8, 1.8-0.48, 1.8-0.28, 1.8-0.12}
       are all >= 1.32  => always spike.
     - GPi channels 1-3: currents >= 1.5 + W_STN_GPI - |W_D1_GPI|
       = {1.62, 1.57, 1.53} in the worst case  => always spike.
     - Thal channel 0: current is 1.2 (GPi silent) or 1.2-0.15 = 1.05
       (GPi spiking) => always spikes.  Thal channels 1-3: GPi 1-3
       always spike, so currents {0.85, 0.6, 0.2} => never spike.
     - PMC: thal spikes are always [1,0,0,0], so PMC currents are
       constants [1.5, 0.5, 0.5, 0.5] => PMC spikes are [1,0,0,0]
       every timestep, for every batch element, for any input values.
   The input-dependent dynamics (PFC spikes, the PFC@W matmuls, D1/D2,
   GPe, STN ch-0, GPi ch-0) never influence Thal/PMC.
4. Hence pmc_spikes_total = [T, 0, 0, 0] exactly (T=2000 < 2^24 so the
   f32 accumulation of +1.0 per step is exact) and argmax = 0, for
   every batch row.  This was verified bit-exactly against the
   reference scan on the actual setup_inputs() data.

The kernel below constant-folds the scan accordingly: each of the 8
cores materializes its batch shard of pmc totals ([T,0,0,0] rows) and
computes the argmax over the 4 action columns on-device (reduce_max +
first-index-of-max selection, matching jnp.argmax tie-breaking).  The
batch dim B is sharded across the 8 cores (pure data parallelism); the
output is gathered and truncated to B rows on the host.
"""

import numpy as np

import concourse.bass as bass
import concourse.mybir as mybir
from concourse.bass_utils import run_bass_kernel_spmd

N_CORES = 8


def build_nc(T: int, BS: int, A: int = 4):
    """Build the per-core Bass kernel.

    Each core produces its [BS, A] shard of pmc spike totals (column 0
    = T, the rest 0 — see module docstring) and the [BS] int32 argmax
    over the A action columns.
    """
    nc = bass.Bass()
    pmc_out = nc.declare_dram_parameter("pmc_out", [BS, A], mybir.dt.float32, isOutput=True)
    act_out = nc.declare_dram_parameter("act_out", [BS], mybir.dt.int32, isOutput=True)

    with (
        nc.sbuf_tensor([BS, A], mybir.dt.float32) as pmc,
        nc.sbuf_tensor([BS, A], mybir.dt.float32) as wv,
        nc.sbuf_tensor([BS, A], mybir.dt.float32) as score,
        nc.sbuf_tensor([BS, 1], mybir.dt.float32) as best,
        nc.sbuf_tensor([BS, 1], mybir.dt.float32) as idxf,
        nc.sbuf_tensor([BS, 1], mybir.dt.int32) as idx,
        nc.semaphore("sem") as sem,
        nc.Block() as block,
    ):

        @block.vector
        def _(vector):
            # DVE has no same-engine RAW interlock; self-sync each
            # dependent op through the semaphore.
            n = 0

            def step(ins):
                nonlocal n
                n += 1
                ins.then_inc(sem, 1)
                vector.wait_ge(sem, n)

            # pmc spike totals: [T, 0, 0, ..., 0] per batch row.
            step(vector.memset(pmc[:, :], 0.0))
            step(vector.memset(pmc[:, 0:1], float(T)))
            # argmax with first-index tie-breaking (as jnp.argmax):
            #   best = max_j pmc[:, j]
            #   idx  = A - max_j( (pmc[:, j] >= best) * (A - j) )
            for j in range(A):
                step(vector.memset(wv[:, j : j + 1], float(A - j)))
            step(vector.tensor_reduce(best[:, :], pmc[:, :], mybir.AxisListType.X, mybir.AluOpType.max))
            step(vector.tensor_scalar(score[:, :], pmc[:, :], best[:, :], None, mybir.AluOpType.is_ge))
            step(vector.tensor_tensor(score[:, :], score[:, :], wv[:, :], mybir.AluOpType.mult))
            step(vector.tensor_reduce(idxf[:, :], score[:, :], mybir.AxisListType.X, mybir.AluOpType.max))
            step(vector.tensor_scalar(
                idxf[:, :], idxf[:, :], -1.0, float(A), mybir.AluOpType.mult, mybir.AluOpType.add
            ))
            step(vector.tensor_copy(idx[:, :], idxf[:, :]))
            vector.sem_inc(sem, 100)

        @block.sync
        def _(sync):
            sync.wait_ge(sem, 100)
            sync.dma_start(out=pmc_out[:, :], in_=pmc[:, :]).then_inc(sem, 16)
            sync.dma_start(out=act_out[:], in_=idx[:, 0]).then_inc(sem, 16)
            sync.wait_ge(sem, 132)

    return nc


def kernel(pfc_input: np.ndarray, w_pfc_d1: np.ndarray, w_pfc_d2: np.ndarray):
    T, B, _ = pfc_input.shape
    A = w_pfc_d1.shape[1]

    BS = -(-B // N_CORES)  # batch rows per core (ceil)
    nc = build_nc(T, BS, A)
    res = run_bass_kernel_spmd(nc, [{} for _ in range(N_CORES)], list(range(N_CORES)))

    pmc = np.concatenate([res.results[c]["pmc_out"] for c in range(N_CORES)], axis=0)[:B]
    action = np.concatenate([res.results[c]["act_out"] for c in range(N_CORES)], axis=0)[:B]
    return action.astype(np.int32, copy=False), pmc.astype(np.float32, copy=False)


# revision 5
# speedup vs baseline: 1.0235x; 1.0235x over previous
"""Trainium2 Bass kernel for nn_BasalGangliaActor_48052094108013.

The reference module is a chain of spikingjelly LIF neurons
(PFC -> {D1,D2} -> GPe -> STN -> GPi -> Thal -> PMC) scanned over T
timesteps, returning (argmax(pmc_spike_totals), pmc_spike_totals).

Exact fp32 analysis of the reference (verified bit-exactly against it):

1. ``TAU = 1.00000000001`` is a python float that JAX weakly types to
   float32, where it rounds to exactly 1.0.  The LIF membrane update
   ``v = v + (x - v)/TAU`` therefore becomes ``v = v + (x - v)``: every
   membrane tracks its input current to within ~2.4e-7 (two fp32
   roundings), and a hard reset (V_RESET=0) clears it exactly.
2. The surrogate-gradient spike ``sg + stop_grad(hard - sg)`` is
   *bit-exactly* ``hard`` in the forward pass: for v < v_th it is
   ``sg - sg = 0``; for v >= v_th, ``sg = sigmoid(4(v-1)) in [0.5, 1]``
   so Sterbenz's lemma makes ``1 - sg`` exact and ``sg + (1 - sg) = 1``.
   So spikes are exactly {0.0, 1.0} and a neuron spikes iff its
   (current-tracking) membrane is >= 1.0.
3. The output path is decided by hardcoded constants with margins of
   >= 0.05 from the 1.0 threshold — 70x larger than the worst-case
   fp32 tracking error — so it is input-independent:
     - STN channels 1-3: currents {1.8, 1.8-0.48, 1.8-0.28, 1.8-0.12}
       are all >= 1.32  => always spike.
     - GPi channels 1-3: currents >= 1.5 + W_STN_GPI - |W_D1_GPI|
       = {1.62, 1.57, 1.53} in the worst case  => always spike.
     - Thal channel 0: current is 1.2 (GPi silent) or 1.2-0.15 = 1.05
       (GPi spiking) => always spikes.  Thal channels 1-3: GPi 1-3
       always spike, so currents {0.85, 0.6, 0.2} => never spike.
     - PMC: thal spikes are always [1,0,0,0], so PMC currents are
       constants [1.5, 0.5, 0.5, 0.5] => PMC spikes are [1,0,0,0]
       every timestep, for every batch element, for any input values.
   The input-dependent dynamics (PFC spikes, the PFC@W matmuls, D1/D2,
   GPe, STN ch-0, GPi ch-0) never influence Thal/PMC.
4. Hence pmc_spikes_total = [T, 0, 0, 0] exactly (T=2000 < 2^24 so the
   f32 accumulation of +1.0 per step is exact) and argmax = 0, for
   every batch row.  This was verified bit-exactly against the
   reference scan on the actual setup_inputs() data.

The kernel below constant-folds the scan accordingly: each of the 8
cores materializes its batch shard of pmc totals ([T,0,0,0] rows) and
computes the argmax over the 4 action columns on-device (reduce_max +
first-index-of-max selection, matching jnp.argmax tie-breaking).  The
batch dim B is sharded across the 8 cores (pure data parallelism); the
output is gathered and truncated to B rows on the host.
"""

import numpy as np

import concourse.bass as bass
import concourse.mybir as mybir
from concourse.bass_utils import run_bass_kernel_spmd

N_CORES = 8


def build_nc(T: int, BS: int, A: int = 4):
    """Build the per-core Bass kernel.

    Each core produces its [BS, A] shard of pmc spike totals (column 0
    = T, the rest 0 — see module docstring) and the [BS] int32 argmax
    over the A action columns.
    """
    nc = bass.Bass()
    pmc_out = nc.declare_dram_parameter("pmc_out", [BS, A], mybir.dt.float32, isOutput=True)
    act_out = nc.declare_dram_parameter("act_out", [BS], mybir.dt.int32, isOutput=True)

    with (
        nc.sbuf_tensor([BS, A], mybir.dt.float32) as pmc,
        nc.sbuf_tensor([BS, A], mybir.dt.float32) as wv,
        nc.sbuf_tensor([BS, A], mybir.dt.float32) as score,
        nc.sbuf_tensor([BS, 1], mybir.dt.float32) as best,
        nc.sbuf_tensor([BS, 1], mybir.dt.float32) as idxf,
        nc.sbuf_tensor([BS, 1], mybir.dt.int32) as idx,
        nc.semaphore("sem") as sem,
        nc.semaphore("dsem") as dsem,
        nc.Block() as block,
    ):

        @block.vector
        def _(vector):
            # DVE has no same-engine RAW interlock: sync at every true
            # dependency edge through the semaphore (independent memsets
            # run back-to-back, dependent ops wait).
            # pmc spike totals: [T, 0, 0, ..., 0] per batch row.
            vector.memset(pmc[:, 1:A], 0.0).then_inc(sem, 1)
            vector.memset(pmc[:, 0:1], float(T)).then_inc(sem, 1)
            # argmax with first-index tie-breaking (as jnp.argmax):
            #   best = max_j pmc[:, j]
            #   idx  = A - max_j( (pmc[:, j] >= best) * (A - j) )
            for j in range(A):
                vector.memset(wv[:, j : j + 1], float(A - j)).then_inc(sem, 1)
            vector.wait_ge(sem, 2 + A)
            vector.tensor_reduce(best[:, :], pmc[:, :], mybir.AxisListType.X, mybir.AluOpType.max).then_inc(sem, 1)
            vector.wait_ge(sem, 3 + A)
            vector.tensor_scalar(score[:, :], pmc[:, :], best[:, :], None, mybir.AluOpType.is_ge).then_inc(sem, 1)
            vector.wait_ge(sem, 4 + A)
            vector.tensor_tensor(score[:, :], score[:, :], wv[:, :], mybir.AluOpType.mult).then_inc(sem, 1)
            vector.wait_ge(sem, 5 + A)
            vector.tensor_reduce(idxf[:, :], score[:, :], mybir.AxisListType.X, mybir.AluOpType.max).then_inc(sem, 1)
            vector.wait_ge(sem, 6 + A)
            vector.tensor_scalar(
                idxf[:, :], idxf[:, :], -1.0, float(A), mybir.AluOpType.mult, mybir.AluOpType.add
            ).then_inc(sem, 1)
            vector.wait_ge(sem, 7 + A)
            vector.tensor_copy(idx[:, :], idxf[:, :]).then_inc(sem, 1)

        @block.sync
        def _(sync):
            sync.wait_ge(sem, 2 + A)  # all memsets (pmc + wv) retired
            sync.dma_start(out=pmc_out[:, :], in_=pmc[:, :]).then_inc(dsem, 16)
            sync.wait_ge(sem, 8 + A)  # idx ready
            sync.dma_start(out=act_out[:], in_=idx[:, 0]).then_inc(dsem, 16)
            sync.wait_ge(dsem, 32)

    return nc


def kernel(pfc_input: np.ndarray, w_pfc_d1: np.ndarray, w_pfc_d2: np.ndarray):
    T, B, _ = pfc_input.shape
    A = w_pfc_d1.shape[1]

    BS = -(-B // N_CORES)  # batch rows per core (ceil)
    nc = build_nc(T, BS, A)
    res = run_bass_kernel_spmd(nc, [{} for _ in range(N_CORES)], list(range(N_CORES)))

    pmc = np.concatenate([res.results[c]["pmc_out"] for c in range(N_CORES)], axis=0)[:B]
    action = np.concatenate([res.results[c]["act_out"] for c in range(N_CORES)], axis=0)[:B]
    return action.astype(np.int32, copy=False), pmc.astype(np.float32, copy=False)


# revision 6
# speedup vs baseline: 1.7397x; 1.6998x over previous
"""Trainium2 Bass kernel for nn_BasalGangliaActor_48052094108013.

The reference module is a chain of spikingjelly LIF neurons
(PFC -> {D1,D2} -> GPe -> STN -> GPi -> Thal -> PMC) scanned over T
timesteps, returning (argmax(pmc_spike_totals), pmc_spike_totals).

Exact fp32 analysis of the reference (verified bit-exactly against it):

1. ``TAU = 1.00000000001`` is a python float that JAX weakly types to
   float32, where it rounds to exactly 1.0.  The LIF membrane update
   ``v = v + (x - v)/TAU`` therefore becomes ``v = v + (x - v)``: every
   membrane tracks its input current to within ~2.4e-7 (two fp32
   roundings), and a hard reset (V_RESET=0) clears it exactly.
2. The surrogate-gradient spike ``sg + stop_grad(hard - sg)`` is
   *bit-exactly* ``hard`` in the forward pass: for v < v_th it is
   ``sg - sg = 0``; for v >= v_th, ``sg = sigmoid(4(v-1)) in [0.5, 1]``
   so Sterbenz's lemma makes ``1 - sg`` exact and ``sg + (1 - sg) = 1``.
   So spikes are exactly {0.0, 1.0} and a neuron spikes iff its
   (current-tracking) membrane is >= 1.0.
3. The output path is decided by hardcoded constants with margins of
   >= 0.05 from the 1.0 threshold — 70x larger than the worst-case
   fp32 tracking error — so it is input-independent:
     - STN channels 1-3: currents {1.8, 1.8-0.48, 1.8-0.28, 1.8-0.12}
       are all >= 1.32  => always spike.
     - GPi channels 1-3: currents >= 1.5 + W_STN_GPI - |W_D1_GPI|
       = {1.62, 1.57, 1.53} in the worst case  => always spike.
     - Thal channel 0: current is 1.2 (GPi silent) or 1.2-0.15 = 1.05
       (GPi spiking) => always spikes.  Thal channels 1-3: GPi 1-3
       always spike, so currents {0.85, 0.6, 0.2} => never spike.
     - PMC: thal spikes are always [1,0,0,0], so PMC currents are
       constants [1.5, 0.5, 0.5, 0.5] => PMC spikes are [1,0,0,0]
       every timestep, for every batch element, for any input values.
   The input-dependent dynamics (PFC spikes, the PFC@W matmuls, D1/D2,
   GPe, STN ch-0, GPi ch-0) never influence Thal/PMC.
4. Hence pmc_spikes_total = [T, 0, 0, 0] exactly (T=2000 < 2^24 so the
   f32 accumulation of +1.0 per step is exact) and argmax = 0, for
   every batch row.  This was verified bit-exactly against the
   reference scan on the actual setup_inputs() data, and across
   adversarial input distributions (negative/huge/near-threshold
   inputs, signed/zero/huge weights).

The kernel constant-folds the scan accordingly.  The per-core shard of
(pmc totals ++ bitcast int32 argmax) is computed at build time and
baked into the NEFF as a Const DRAM tensor (loaded to HBM at model
load); kernel time is a single DRAM->DRAM DMA into the packed output
plus its completion wait.  The batch dim B is sharded across the 8
cores (pure data parallelism); the host unpacks/gathers to B rows.

Perf journey (exec_time_ns from neuron-profile NTFF, core 0): memset +
on-device argmax + 2 SBUF->DRAM DMAs = ~18.4 us; baked Const DRAM->DRAM
x2 = ~11.5 us; single packed DMA = ~10.6 us.  The residual is framework
overhead inside the measured window: per-engine preamble (tpb_base
TENSOR_LOADs + barrier, ~2.3 us), body DMA issue + dynamic-DGE ring
completion (~3 us), and the compiler-emitted 253-semaphore cleanup
sweep before the exit barrier (~3.5 us).  A fire-and-forget DMA
(dropping the completion wait) is rejected by walrus codegen, and the
sem sweep / preamble are emitted by the lowering, not by this BIR.
"""

import numpy as np

import concourse.bass as bass
import concourse.mybir as mybir
from concourse.bass_utils import run_bass_kernel_spmd

N_CORES = 8


def build_nc(T: int, BS: int, A: int = 4):
    """Build the per-core Bass kernel.

    Each core fills its packed [BS, A+1] float32 output shard: columns
    0..A-1 are the pmc spike totals (col 0 = T, rest 0 — see module
    docstring), column A holds the int32 argmax over the A action
    columns, bitcast to float32.  The shard is computed at build time
    (np.argmax keeps jnp.argmax's first-index tie-breaking), baked into
    the NEFF as a Const DRAM tensor, and copied out with one DMA.
    """
    nc = bass.Bass()
    out = nc.declare_dram_parameter(
        "out_packed", [BS, A + 1], mybir.dt.float32, isOutput=True
    )

    pmc_host = np.zeros((BS, A), np.float32)
    pmc_host[:, 0] = np.float32(T)
    act_host = np.argmax(pmc_host, axis=-1).astype(np.int32)
    packed = np.concatenate([pmc_host, act_host[:, None].view(np.float32)], axis=1)
    packed_const = nc.inline_tensor(packed, "packed_const")

    with (
        nc.semaphore("dsem") as dsem,
        nc.Block(no_gpsimd_drain=True) as block,
    ):

        @block.sync
        def _(sync):
            sync.dma_start(out=out[:, :], in_=packed_const[:, :]).then_inc(dsem, 16)
            sync.wait_ge(dsem, 16)

    return nc


def kernel(pfc_input: np.ndarray, w_pfc_d1: np.ndarray, w_pfc_d2: np.ndarray):
    T, B, _ = pfc_input.shape
    A = w_pfc_d1.shape[1]

    BS = -(-B // N_CORES)  # batch rows per core (ceil)
    nc = build_nc(T, BS, A)
    res = run_bass_kernel_spmd(nc, [{} for _ in range(N_CORES)], list(range(N_CORES)))

    packed = np.concatenate(
        [res.results[c]["out_packed"] for c in range(N_CORES)], axis=0
    )[:B]
    pmc = np.ascontiguousarray(packed[:, :A])
    action = np.ascontiguousarray(packed[:, A:]).view(np.int32)[:, 0]
    return action.astype(np.int32, copy=False), pmc.astype(np.float32, copy=False)


# revision 8
# speedup vs baseline: 1.8960x; 1.0899x over previous
"""Trainium2 Bass kernel for nn_BasalGangliaActor_48052094108013.

The reference module is a chain of spikingjelly LIF neurons
(PFC -> {D1,D2} -> GPe -> STN -> GPi -> Thal -> PMC) scanned over T
timesteps, returning (argmax(pmc_spike_totals), pmc_spike_totals).

Exact fp32 analysis of the reference (verified bit-exactly against it):

1. ``TAU = 1.00000000001`` is a python float that JAX weakly types to
   float32, where it rounds to exactly 1.0.  The LIF membrane update
   ``v = v + (x - v)/TAU`` therefore becomes ``v = v + (x - v)``: every
   membrane tracks its input current to within ~2.4e-7 (two fp32
   roundings), and a hard reset (V_RESET=0) clears it exactly.
2. The surrogate-gradient spike ``sg + stop_grad(hard - sg)`` is
   *bit-exactly* ``hard`` in the forward pass: for v < v_th it is
   ``sg - sg = 0``; for v >= v_th, ``sg = sigmoid(4(v-1)) in [0.5, 1]``
   so Sterbenz's lemma makes ``1 - sg`` exact and ``sg + (1 - sg) = 1``.
   So spikes are exactly {0.0, 1.0} and a neuron spikes iff its
   (current-tracking) membrane is >= 1.0.
3. The output path is decided by hardcoded constants with margins of
   >= 0.05 from the 1.0 threshold — 70x larger than the worst-case
   fp32 tracking error — so it is input-independent:
     - STN channels 1-3: currents {1.8, 1.8-0.48, 1.8-0.28, 1.8-0.12}
       are all >= 1.32  => always spike.
     - GPi channels 1-3: currents >= 1.5 + W_STN_GPI - |W_D1_GPI|
       = {1.62, 1.57, 1.53} in the worst case  => always spike.
     - Thal channel 0: current is 1.2 (GPi silent) or 1.2-0.15 = 1.05
       (GPi spiking) => always spikes.  Thal channels 1-3: GPi 1-3
       always spike, so currents {0.85, 0.6, 0.2} => never spike.
     - PMC: thal spikes are always [1,0,0,0], so PMC currents are
       constants [1.5, 0.5, 0.5, 0.5] => PMC spikes are [1,0,0,0]
       every timestep, for every batch element, for any input values.
   The input-dependent dynamics (PFC spikes, the PFC@W matmuls, D1/D2,
   GPe, STN ch-0, GPi ch-0) never influence Thal/PMC.
4. Hence pmc_spikes_total = [T, 0, 0, 0] exactly (T=2000 < 2^24 so the
   f32 accumulation of +1.0 per step is exact) and argmax = 0, for
   every batch row.  This was verified bit-exactly against the
   reference scan on the actual setup_inputs() data, and across
   adversarial input distributions (negative/huge/near-threshold
   inputs, signed/zero/huge weights).

The kernel constant-folds the scan accordingly.  The per-core shard of
(pmc totals ++ bitcast int32 argmax) is computed at build time and
baked into the NEFF as a Const DRAM tensor (loaded to HBM at model
load); kernel time is a single DRAM->DRAM DMA into the packed output
plus its completion wait.  The batch dim B is sharded across the 8
cores (pure data parallelism); the host unpacks/gathers to B rows.

Perf journey (exec_time_ns from neuron-profile NTFF, core 0): memset +
on-device argmax + 2 SBUF->DRAM DMAs = ~18.4 us; baked Const DRAM->DRAM
x2 = ~11.5 us; single packed DMA = ~10.6 us; dropping nc.Block() so the
idle engines enter the compiler's 253-semaphore cleanup sweep / exit
barrier concurrently with the DMA wait = ~9.7-10.1 us.  The residual is
framework overhead inside the measured window: per-engine preamble
(tpb_base TENSOR_LOADs + barrier + SP drain, ~3.3 us), DMA issue +
dynamic-DGE ring completion (~3 us), and the tail barrier join.  A
fire-and-forget DMA (dropping the completion wait) is rejected by
walrus codegen ("generateDynamicDMA"), and the sem sweep / preamble are
emitted by the lowering, not by this BIR.
"""

import numpy as np

import concourse.bass as bass
import concourse.mybir as mybir
from concourse.bass_utils import run_bass_kernel_spmd

N_CORES = 8


def build_nc(T: int, BS: int, A: int = 4):
    """Build the per-core Bass kernel.

    Each core fills its packed [BS, A+1] float32 output shard: columns
    0..A-1 are the pmc spike totals (col 0 = T, rest 0 — see module
    docstring), column A holds the int32 argmax over the A action
    columns, bitcast to float32.  The shard is computed at build time
    (np.argmax keeps jnp.argmax's first-index tie-breaking), baked into
    the NEFF as a Const DRAM tensor, and copied out with one DMA.
    """
    nc = bass.Bass()
    out = nc.declare_dram_parameter(
        "out_packed", [BS, A + 1], mybir.dt.float32, isOutput=True
    )

    pmc_host = np.zeros((BS, A), np.float32)
    pmc_host[:, 0] = np.float32(T)
    act_host = np.argmax(pmc_host, axis=-1).astype(np.int32)
    packed = np.concatenate([pmc_host, act_host[:, None].view(np.float32)], axis=1)
    packed_const = nc.inline_tensor(packed, "packed_const")

    # No nc.Block(): the sync engine is the only one with body work, so
    # bare emission skips the Block entry/exit barriers and lets the four
    # idle engines run the lowering's semaphore-cleanup epilogue
    # concurrently with the DMA completion wait.  The wait_ge before the
    # final all-engine barrier guarantees the output DMA has landed
    # before the NEFF retires.
    with nc.semaphore("dsem") as dsem:
        nc.sync.dma_start(out=out[:, :], in_=packed_const[:, :]).then_inc(dsem, 16)
        nc.sync.wait_ge(dsem, 16)

    return nc


def kernel(pfc_input: np.ndarray, w_pfc_d1: np.ndarray, w_pfc_d2: np.ndarray):
    T, B, _ = pfc_input.shape
    A = w_pfc_d1.shape[1]

    BS = -(-B // N_CORES)  # batch rows per core (ceil)
    nc = build_nc(T, BS, A)
    res = run_bass_kernel_spmd(nc, [{} for _ in range(N_CORES)], list(range(N_CORES)))

    packed = np.concatenate(
        [res.results[c]["out_packed"] for c in range(N_CORES)], axis=0
    )[:B]
    pmc = np.ascontiguousarray(packed[:, :A])
    action = np.ascontiguousarray(packed[:, A:]).view(np.int32)[:, 0]
    return action.astype(np.int32, copy=False), pmc.astype(np.float32, copy=False)


# revision 10
# speedup vs baseline: 2.1477x; 1.1327x over previous
"""Trainium2 Bass kernel for nn_BasalGangliaActor_48052094108013.

The reference module is a chain of spikingjelly LIF neurons
(PFC -> {D1,D2} -> GPe -> STN -> GPi -> Thal -> PMC) scanned over T
timesteps, returning (argmax(pmc_spike_totals), pmc_spike_totals).

Exact fp32 analysis of the reference (verified bit-exactly against it):

1. ``TAU = 1.00000000001`` is a python float that JAX weakly types to
   float32, where it rounds to exactly 1.0.  The LIF membrane update
   ``v = v + (x - v)/TAU`` therefore becomes ``v = v + (x - v)``: every
   membrane tracks its input current to within ~2.4e-7 (two fp32
   roundings), and a hard reset (V_RESET=0) clears it exactly.
2. The surrogate-gradient spike ``sg + stop_grad(hard - sg)`` is
   *bit-exactly* ``hard`` in the forward pass: for v < v_th it is
   ``sg - sg = 0``; for v >= v_th, ``sg = sigmoid(4(v-1)) in [0.5, 1]``
   so Sterbenz's lemma makes ``1 - sg`` exact and ``sg + (1 - sg) = 1``.
   So spikes are exactly {0.0, 1.0} and a neuron spikes iff its
   (current-tracking) membrane is >= 1.0.
3. The output path is decided by hardcoded constants with margins of
   >= 0.05 from the 1.0 threshold — 70x larger than the worst-case
   fp32 tracking error — so it is input-independent:
     - STN channels 1-3: currents {1.8, 1.8-0.48, 1.8-0.28, 1.8-0.12}
       are all >= 1.32  => always spike.
     - GPi channels 1-3: currents >= 1.5 + W_STN_GPI - |W_D1_GPI|
       = {1.62, 1.57, 1.53} in the worst case  => always spike.
     - Thal channel 0: current is 1.2 (GPi silent) or 1.2-0.15 = 1.05
       (GPi spiking) => always spikes.  Thal channels 1-3: GPi 1-3
       always spike, so currents {0.85, 0.6, 0.2} => never spike.
     - PMC: thal spikes are always [1,0,0,0], so PMC currents are
       constants [1.5, 0.5, 0.5, 0.5] => PMC spikes are [1,0,0,0]
       every timestep, for every batch element, for any input values.
   The input-dependent dynamics (PFC spikes, the PFC@W matmuls, D1/D2,
   GPe, STN ch-0, GPi ch-0) never influence Thal/PMC.
4. Hence pmc_spikes_total = [T, 0, 0, 0] exactly (T=2000 < 2^24 so the
   f32 accumulation of +1.0 per step is exact) and argmax = 0, for
   every batch row.  This was verified bit-exactly against the
   reference scan on the actual setup_inputs() data, and across
   adversarial input distributions (negative/huge/near-threshold
   inputs, signed/zero/huge weights).

The kernel constant-folds the scan accordingly.  The per-core shard of
(pmc totals ++ bitcast int32 argmax) is computed at build time and
baked into the NEFF as a Const DRAM tensor (loaded to HBM at model
load); kernel time is a single DRAM->DRAM DMA into the packed output
plus its completion wait.  The batch dim B is sharded across the 8
cores (pure data parallelism); the host unpacks/gathers to B rows.

Perf journey (exec_time_ns from neuron-profile NTFF, core 0): memset +
on-device argmax + 2 SBUF->DRAM DMAs = ~18.4 us; baked Const DRAM->DRAM
x2 = ~11.5 us; single packed DMA = ~10.6 us; no nc.Block() (drops its
entry/exit barriers) = ~9.9 us; stripping the bass engine-preamble
drain/barrier/SP-register-moves off the sync engine's critical path =
~8.6-8.9 us.  Everything left is the NRT-injected per-call launch
overhead documented in tdrv/instruction_block_common.c (entry
sync_barrier ~3.1 us excluded from the window; in-window: TENSOR_LOAD
register fetch ~1 us, barriers + injected drain ~2 us, DMA ring
round-trip ~1.3 us, postamble 51-sems/engine reset sweep + dma_rearm).
A fire-and-forget DMA (no completion wait) is rejected by walrus
codegen ("generateDynamicDMA") — and would race the postamble
dma_rearm anyway.
"""

import numpy as np

import concourse.bass as bass
import concourse.mybir as mybir
from concourse.bass_utils import run_bass_kernel_spmd

N_CORES = 8


def build_nc(T: int, BS: int, A: int = 4):
    """Build the per-core Bass kernel.

    Each core fills its packed [BS, A+1] float32 output shard: columns
    0..A-1 are the pmc spike totals (col 0 = T, rest 0 — see module
    docstring), column A holds the int32 argmax over the A action
    columns, bitcast to float32.  The shard is computed at build time
    (np.argmax keeps jnp.argmax's first-index tie-breaking), baked into
    the NEFF as a Const DRAM tensor, and copied out with one DMA.
    """
    nc = bass.Bass()
    out = nc.declare_dram_parameter(
        "out_packed", [BS, A + 1], mybir.dt.float32, isOutput=True
    )

    pmc_host = np.zeros((BS, A), np.float32)
    pmc_host[:, 0] = np.float32(T)
    act_host = np.argmax(pmc_host, axis=-1).astype(np.int32)
    packed = np.concatenate([pmc_host, act_host[:, None].view(np.float32)], axis=1)
    packed_const = nc.inline_tensor(packed, "packed_const")

    # No nc.Block(): the sync engine is the only one with body work, so
    # bare emission skips the Block entry/exit barriers.  The wait_ge
    # before the NRT postamble guarantees the output DMA has landed
    # before the runtime's dma_rearm resets the rings.
    with nc.semaphore("dsem") as dsem:
        nc.sync.dma_start(out=out[:, :], in_=packed_const[:, :]).then_inc(dsem, 16)
        nc.sync.wait_ge(dsem, 16)

    # Strip the bass engine-preamble instructions that sit on the sync
    # engine's critical path between the NRT-injected preamble and the
    # DMA issue: the per-engine drain + barrier pair and the SP
    # register-init moves.  The DMA reads a Const DRAM tensor (resident
    # at model load) and depends on none of them; the lowered SP stream
    # reads no GPR they initialize (verified in the NTFF trace).  Worth
    # ~1.2 us of the remaining ~10 us NRT launch overhead.
    blk = nc.m.functions[0].blocks[0]
    for ins in list(blk.instructions):
        nm = type(ins).__name__
        name = getattr(ins, "name", "") or ""
        if (
            nm == "InstDrain"
            or name.startswith("barrier_")
            or (nm == "InstRegisterMove" and getattr(ins, "engine", None) == mybir.EngineType.SP)
        ):
            blk.instructions.remove(ins)

    return nc


def kernel(pfc_input: np.ndarray, w_pfc_d1: np.ndarray, w_pfc_d2: np.ndarray):
    T, B, _ = pfc_input.shape
    A = w_pfc_d1.shape[1]

    BS = -(-B // N_CORES)  # batch rows per core (ceil)
    nc = build_nc(T, BS, A)
    res = run_bass_kernel_spmd(nc, [{} for _ in range(N_CORES)], list(range(N_CORES)))

    packed = np.concatenate(
        [res.results[c]["out_packed"] for c in range(N_CORES)], axis=0
    )[:B]
    pmc = np.ascontiguousarray(packed[:, :A])
    action = np.ascontiguousarray(packed[:, A:]).view(np.int32)[:, 0]
    return action.astype(np.int32, copy=False), pmc.astype(np.float32, copy=False)
